# revision 6
# baseline (speedup 1.0000x reference)
"""Trainium2 Bass kernel for nn_Pre_loss_53566832116190 (topk_masking).

Strategy: data-parallel over batch N across 8 NeuronCores. Each core
computes the per-row KL criterion for its shard of rows for both the
(pred_x, gt_x) and (pred_y, gt_y) pairs:

    loss[r] = ( S/s_l - ln(s_l) + ln(s_p) ) / W
      with e_l = exp(l), s_l = sum(e_l), s_p = sum(exp(p)),
           S = sum(e_l * (l - p))

which is algebraically KLDiv(log_softmax(p), softmax(l)).mean(axis=-1).
The small [N*K] merge (global top-k mask, weights, weighted loss sum)
is done on the host after gathering per-core loss rows.
"""

import numpy as np

_N, _K, _W = 1024, 17, 512
_NCORES = 8
_NB = _N // _NCORES      # 128 batch rows per core
_M = _NB * _K            # 2176 loss rows per core
_T = _M // 128           # 17 row-tiles of 128 rows
_COLS = 2 * _T           # x tiles then y tiles

_nc_cache = None
last_results = None  # BassKernelResults of the most recent run (for test harness)


def _build_module():
    import concourse.bacc as bacc
    import concourse.mybir as mybir
    import concourse.tile as tile

    f32 = mybir.dt.float32
    Exp = mybir.ActivationFunctionType.Exp
    Ln = mybir.ActivationFunctionType.Ln
    mult = mybir.AluOpType.mult
    add = mybir.AluOpType.add

    # Bacc (not plain Bass): its finalize() runs generate_event_semaphores,
    # which splits multi-wait sync conditions that TRN2 structs can't hold.
    nc = bacc.Bacc("TRN2", debug=False)

    dram = {}
    for name in ("pred_x", "gt_x", "pred_y", "gt_y"):
        dram[name] = nc.dram_tensor(name, [_M, _W], f32, kind="ExternalInput").ap()
    out_d = nc.dram_tensor("loss_out", [128, _COLS], f32, kind="ExternalOutput").ap()

    # Two DMA groups per pair: 9 + 8 row-tiles. Per (group, tensor) the
    # exp row-sums either come from per-row ACT accumulators ("acc") or
    # from one grouped DVE reduce ("grp") — mix chosen to balance the ACT
    # and DVE engines.
    groups = [(0, 9), (9, 8)]
    modes = {  # (group_index, tensor): "acc" | "grp"
        (0, "l"): "acc",
        (0, "p"): "grp",
        (1, "l"): "grp",
        (1, "p"): "grp",
    }

    with tile.TileContext(nc) as tc:
        with (
            tc.tile_pool(name="io", bufs=2) as io_pool,
            tc.tile_pool(name="work", bufs=2) as work_pool,
            tc.tile_pool(name="acc", bufs=1) as acc_pool,
        ):
            sl_acc = acc_pool.tile([128, _COLS], f32)  # sum exp(l)
            sp_acc = acc_pool.tile([128, _COLS], f32)  # sum exp(p)
            ss_acc = acc_pool.tile([128, _COLS], f32)  # sum e_l*(l-p)

            pairs = (("pred_x", "gt_x"), ("pred_y", "gt_y"))
            for pi, (pname, lname) in enumerate(pairs):
                # row r = t*128 + p  ->  [p, t, w]
                p_r = dram[pname].rearrange("(t p) w -> p t w", p=128)
                l_r = dram[lname].rearrange("(t p) w -> p t w", p=128)
                for gi, (t0, gs) in enumerate(groups):
                    lt = io_pool.tile([128, gs, _W], f32, tag="lt")
                    nc.sync.dma_start(out=lt, in_=l_r[:, t0 : t0 + gs, :])
                    pt = io_pool.tile([128, gs, _W], f32, tag="pt")
                    nc.sync.dma_start(out=pt, in_=p_r[:, t0 : t0 + gs, :])

                    # d = l - p on the (otherwise idle) GpSimd engine
                    dt_ = work_pool.tile([128, gs, _W], f32, tag="dt")
                    nc.gpsimd.tensor_sub(dt_, lt, pt)

                    col = pi * _T + t0
                    # exp in place; row sums per chosen mode
                    for tens, tilebuf, acc in (
                        ("l", lt, sl_acc),
                        ("p", pt, sp_acc),
                    ):
                        if modes[(gi, tens)] == "acc":
                            for j in range(gs):
                                nc.scalar.activation(
                                    tilebuf[:, j, :], tilebuf[:, j, :], Exp,
                                    accum_out=acc[:, col + j : col + j + 1],
                                )
                        else:
                            nc.scalar.activation(tilebuf, tilebuf, Exp)
                            nc.vector.tensor_reduce(
                                out=acc[:, col : col + gs],
                                in_=tilebuf,
                                axis=mybir.AxisListType.X,
                                op=add,
                            )

                    # ss[r] = sum(e_l * d) fused in one DVE pass per row
                    # tile; product scratch overwrites pt (e_p consumed)
                    for j in range(gs):
                        nc.vector.affine_mul_reduce(
                            out=pt[:, j, :],
                            accum_out=ss_acc[:, col + j : col + j + 1],
                            in0=dt_[:, j, :],
                            in1=lt[:, j, :],
                            scale=1.0,
                            bias=0.0,
                        )

            res_t = acc_pool.tile([128, _COLS], f32)
            nc.vector.reciprocal(res_t, sl_acc)           # 1/s_l
            nc.vector.tensor_mul(res_t, ss_acc, res_t)    # S/s_l
            ln_sl = acc_pool.tile([128, _COLS], f32)
            nc.scalar.activation(ln_sl, sl_acc, Ln)
            ln_sp = acc_pool.tile([128, _COLS], f32)
            nc.scalar.activation(ln_sp, sp_acc, Ln)
            nc.vector.tensor_sub(res_t, res_t, ln_sl)
            nc.vector.tensor_add(res_t, res_t, ln_sp)
            out_t = acc_pool.tile([128, _COLS], f32)
            nc.scalar.mul(out_t, res_t, 1.0 / _W)
            nc.sync.dma_start(out=out_d, in_=out_t)

    nc.finalize()
    return nc


def get_module():
    global _nc_cache
    if _nc_cache is None:
        _nc_cache = _build_module()
    return _nc_cache


def _host_finish(loss_x, loss_y, target_weight, use_labels, epoch):
    """Replicates reference's cheap [N*K]-sized tail exactly (numpy)."""
    tw = np.asarray(target_weight, dtype=np.float32)
    ul = np.asarray(use_labels)
    weight_real = (tw * (ul == 0).astype(np.float32)[:, None]).reshape(-1)

    cur = float(np.clip(int(epoch) - 210, 0.0, 30.0))
    r = 0.5 * (np.cos(np.pi * cur / 30.0) + 1.0)
    rate = float(np.clip(r, 0.8, 1.0))
    num_visible = int(np.count_nonzero(tw))
    k = int(num_visible * rate)

    tw_flat = tw.reshape(-1)
    loss_all = 0.0
    weights = []
    for loss_small in (loss_x, loss_y):
        loss_new = np.where(tw_flat > 0, loss_small, loss_small.max())
        # k smallest values; ties broken toward lower index (matches
        # jax.lax.top_k on the negated vector)
        idx = np.argsort(loss_new, kind="stable")[:k]
        mask = np.zeros(_N * _K, dtype=np.float32)
        mask[idx] = 1.0
        weight_all = (np.float32(2.0) * weight_real + mask).astype(np.float32)
        weights.append(weight_all)
        loss_all += float(
            np.sum(loss_small.astype(np.float64) * weight_all.astype(np.float64))
        )
    loss = np.float32(loss_all / _K)
    return (np.asarray(loss, dtype=np.float32), weights[0], weights[1])


def _ensure_axon_hooks_importable():
    # concourse.bass_utils imports antenv.axon_hooks when BASS_TRACE is set
    # under axon; some containers ship an antenv stub without it.
    try:
        import antenv.axon_hooks  # noqa: F401
    except Exception:
        import sys
        import types

        m = types.ModuleType("antenv.axon_hooks")
        m._hook = None
        m.set_axon_ntff_profile_hook = lambda hook: setattr(m, "_hook", hook)
        m.get_axon_ntff_profile_hook = lambda: m._hook
        sys.modules["antenv.axon_hooks"] = m


def kernel(pred_x, pred_y, gt_x, gt_y, target_weight, use_labels, epoch):
    global last_results
    _ensure_axon_hooks_importable()
    from concourse import bass_utils

    pred_x = np.ascontiguousarray(np.asarray(pred_x, dtype=np.float32))
    pred_y = np.ascontiguousarray(np.asarray(pred_y, dtype=np.float32))
    gt_x = np.ascontiguousarray(np.asarray(gt_x, dtype=np.float32))
    gt_y = np.ascontiguousarray(np.asarray(gt_y, dtype=np.float32))

    nc = get_module()
    in_maps = []
    for c in range(_NCORES):
        s = slice(c * _NB, (c + 1) * _NB)
        in_maps.append(
            {
                "pred_x": np.ascontiguousarray(pred_x[s].reshape(_M, _W)),
                "gt_x": np.ascontiguousarray(gt_x[s].reshape(_M, _W)),
                "pred_y": np.ascontiguousarray(pred_y[s].reshape(_M, _W)),
                "gt_y": np.ascontiguousarray(gt_y[s].reshape(_M, _W)),
            }
        )

    res = bass_utils.run_bass_kernel_spmd(nc, in_maps, core_ids=list(range(_NCORES)))
    last_results = res

    loss_x = np.empty((_NCORES, _T, 128), dtype=np.float32)
    loss_y = np.empty((_NCORES, _T, 128), dtype=np.float32)
    for c, r in enumerate(res.results):
        o = r["loss_out"]  # [128, 2T]; [p, t] = row t*128+p of this shard
        loss_x[c] = o[:, :_T].T
        loss_y[c] = o[:, _T:].T

    return _host_finish(
        loss_x.reshape(-1), loss_y.reshape(-1), target_weight, use_labels, epoch
    )


# revision 7
# speedup vs baseline: 1.4389x; 1.4389x over previous
"""Trainium2 Bass kernel for nn_Pre_loss_53566832116190 (topk_masking).

Strategy: data-parallel over batch N across 8 NeuronCores. Each core
computes the per-row KL criterion for its shard of rows for both the
(pred_x, gt_x) and (pred_y, gt_y) pairs:

    loss[r] = ( S/s_l - ln(s_l) + ln(s_p) ) / W
      with e_l = exp(l), s_l = sum(e_l), s_p = sum(exp(p)),
           S = sum(e_l * (l - p))

which is algebraically KLDiv(log_softmax(p), softmax(l)).mean(axis=-1).
The small [N*K] merge (global top-k mask, weights, weighted loss sum)
is done on the host after gathering per-core loss rows.
"""

import numpy as np

_N, _K, _W = 1024, 17, 512
_NCORES = 8
_NB = _N // _NCORES      # 128 batch rows per core
_M = _NB * _K            # 2176 loss rows per core
_T = _M // 128           # 17 row-tiles of 128 rows
_COLS = 2 * _T           # x tiles then y tiles

_nc_cache = None
last_results = None  # BassKernelResults of the most recent run (for test harness)


def _build_module():
    import concourse.bacc as bacc
    import concourse.mybir as mybir
    import concourse.tile as tile

    f32 = mybir.dt.float32
    Exp = mybir.ActivationFunctionType.Exp
    Ln = mybir.ActivationFunctionType.Ln
    mult = mybir.AluOpType.mult
    add = mybir.AluOpType.add

    # Bacc (not plain Bass): its finalize() runs generate_event_semaphores,
    # which splits multi-wait sync conditions that TRN2 structs can't hold.
    nc = bacc.Bacc("TRN2", debug=False)

    dram = {}
    for name in ("pred_x", "gt_x", "pred_y", "gt_y"):
        dram[name] = nc.dram_tensor(name, [_M, _W], f32, kind="ExternalInput").ap()
    out_d = nc.dram_tensor("loss_out", [128, _COLS], f32, kind="ExternalOutput").ap()

    # Row-tile groups of up to 4 (1 MiB DMA per tensor per group). Per
    # (group, tensor) the exp row-sums either come from per-row ACT
    # accumulators ("acc") or one grouped DVE reduce ("grp") — the mix
    # balances the ACT and DVE engines.
    groups = []
    t0 = 0
    while t0 < _T:
        gs = min(4, _T - t0)
        groups.append((t0, gs))
        t0 += gs

    def l_mode(gi):
        return "acc" if gi < 3 else "grp"  # 12 of 17 l-tiles per pair: acc

    with tile.TileContext(nc) as tc:
        with (
            tc.tile_pool(name="io", bufs=3) as io_pool,
            tc.tile_pool(name="work", bufs=3) as work_pool,
            tc.tile_pool(name="acc", bufs=1) as acc_pool,
        ):
            sl_acc = acc_pool.tile([128, _COLS], f32)  # sum exp(l)
            sp_acc = acc_pool.tile([128, _COLS], f32)  # sum exp(p)
            ss_acc = acc_pool.tile([128, _COLS], f32)  # sum e_l*(l-p)

            pairs = (("pred_x", "gt_x"), ("pred_y", "gt_y"))
            for pi, (pname, lname) in enumerate(pairs):
                # row r = t*128 + p  ->  [p, t, w]
                p_r = dram[pname].rearrange("(t p) w -> p t w", p=128)
                l_r = dram[lname].rearrange("(t p) w -> p t w", p=128)
                for gi, (t0, gs) in enumerate(groups):
                    lt = io_pool.tile([128, gs, _W], f32, tag="lt")
                    nc.sync.dma_start(out=lt, in_=l_r[:, t0 : t0 + gs, :])
                    pt = io_pool.tile([128, gs, _W], f32, tag="pt")
                    nc.sync.dma_start(out=pt, in_=p_r[:, t0 : t0 + gs, :])

                    # d = l - p on the (otherwise idle) GpSimd engine,
                    # in parallel with the exps below
                    dt_ = work_pool.tile([128, gs, _W], f32, tag="dt")
                    nc.gpsimd.tensor_sub(dt_, lt, pt)

                    col = pi * _T + t0
                    el = work_pool.tile([128, gs, _W], f32, tag="el")
                    if l_mode(gi) == "acc":
                        for j in range(gs):
                            nc.scalar.activation(
                                el[:, j, :], lt[:, j, :], Exp,
                                accum_out=sl_acc[:, col + j : col + j + 1],
                            )
                    else:
                        nc.scalar.activation(el, lt, Exp)
                        nc.vector.tensor_reduce(
                            out=sl_acc[:, col : col + gs],
                            in_=el,
                            axis=mybir.AxisListType.X,
                            op=add,
                        )
                    ep = work_pool.tile([128, gs, _W], f32, tag="ep")
                    nc.scalar.activation(ep, pt, Exp)
                    nc.vector.tensor_reduce(
                        out=sp_acc[:, col : col + gs],
                        in_=ep,
                        axis=mybir.AxisListType.X,
                        op=add,
                    )

                    # ss[r] = sum(e_l * d) fused in one DVE pass per row
                    # tile; product scratch overwrites ep (e_p consumed)
                    for j in range(gs):
                        nc.vector.affine_mul_reduce(
                            out=ep[:, j, :],
                            accum_out=ss_acc[:, col + j : col + j + 1],
                            in0=dt_[:, j, :],
                            in1=el[:, j, :],
                            scale=1.0,
                            bias=0.0,
                        )

            res_t = acc_pool.tile([128, _COLS], f32)
            nc.vector.reciprocal(res_t, sl_acc)           # 1/s_l
            nc.vector.tensor_mul(res_t, ss_acc, res_t)    # S/s_l
            ln_sl = acc_pool.tile([128, _COLS], f32)
            nc.scalar.activation(ln_sl, sl_acc, Ln)
            ln_sp = acc_pool.tile([128, _COLS], f32)
            nc.scalar.activation(ln_sp, sp_acc, Ln)
            nc.vector.tensor_sub(res_t, res_t, ln_sl)
            nc.vector.tensor_add(res_t, res_t, ln_sp)
            out_t = acc_pool.tile([128, _COLS], f32)
            nc.scalar.mul(out_t, res_t, 1.0 / _W)
            nc.sync.dma_start(out=out_d, in_=out_t)

    nc.finalize()
    return nc


def get_module():
    global _nc_cache
    if _nc_cache is None:
        _nc_cache = _build_module()
    return _nc_cache


def _host_finish(loss_x, loss_y, target_weight, use_labels, epoch):
    """Replicates reference's cheap [N*K]-sized tail exactly (numpy)."""
    tw = np.asarray(target_weight, dtype=np.float32)
    ul = np.asarray(use_labels)
    weight_real = (tw * (ul == 0).astype(np.float32)[:, None]).reshape(-1)

    cur = float(np.clip(int(epoch) - 210, 0.0, 30.0))
    r = 0.5 * (np.cos(np.pi * cur / 30.0) + 1.0)
    rate = float(np.clip(r, 0.8, 1.0))
    num_visible = int(np.count_nonzero(tw))
    k = int(num_visible * rate)

    tw_flat = tw.reshape(-1)
    loss_all = 0.0
    weights = []
    for loss_small in (loss_x, loss_y):
        loss_new = np.where(tw_flat > 0, loss_small, loss_small.max())
        # k smallest values; ties broken toward lower index (matches
        # jax.lax.top_k on the negated vector)
        idx = np.argsort(loss_new, kind="stable")[:k]
        mask = np.zeros(_N * _K, dtype=np.float32)
        mask[idx] = 1.0
        weight_all = (np.float32(2.0) * weight_real + mask).astype(np.float32)
        weights.append(weight_all)
        loss_all += float(
            np.sum(loss_small.astype(np.float64) * weight_all.astype(np.float64))
        )
    loss = np.float32(loss_all / _K)
    return (np.asarray(loss, dtype=np.float32), weights[0], weights[1])


def _ensure_axon_hooks_importable():
    # concourse.bass_utils imports antenv.axon_hooks when BASS_TRACE is set
    # under axon; some containers ship an antenv stub without it.
    try:
        import antenv.axon_hooks  # noqa: F401
    except Exception:
        import sys
        import types

        m = types.ModuleType("antenv.axon_hooks")
        m._hook = None
        m.set_axon_ntff_profile_hook = lambda hook: setattr(m, "_hook", hook)
        m.get_axon_ntff_profile_hook = lambda: m._hook
        sys.modules["antenv.axon_hooks"] = m


def kernel(pred_x, pred_y, gt_x, gt_y, target_weight, use_labels, epoch):
    global last_results
    _ensure_axon_hooks_importable()
    from concourse import bass_utils

    pred_x = np.ascontiguousarray(np.asarray(pred_x, dtype=np.float32))
    pred_y = np.ascontiguousarray(np.asarray(pred_y, dtype=np.float32))
    gt_x = np.ascontiguousarray(np.asarray(gt_x, dtype=np.float32))
    gt_y = np.ascontiguousarray(np.asarray(gt_y, dtype=np.float32))

    nc = get_module()
    in_maps = []
    for c in range(_NCORES):
        s = slice(c * _NB, (c + 1) * _NB)
        in_maps.append(
            {
                "pred_x": np.ascontiguousarray(pred_x[s].reshape(_M, _W)),
                "gt_x": np.ascontiguousarray(gt_x[s].reshape(_M, _W)),
                "pred_y": np.ascontiguousarray(pred_y[s].reshape(_M, _W)),
                "gt_y": np.ascontiguousarray(gt_y[s].reshape(_M, _W)),
            }
        )

    res = bass_utils.run_bass_kernel_spmd(nc, in_maps, core_ids=list(range(_NCORES)))
    last_results = res

    loss_x = np.empty((_NCORES, _T, 128), dtype=np.float32)
    loss_y = np.empty((_NCORES, _T, 128), dtype=np.float32)
    for c, r in enumerate(res.results):
        o = r["loss_out"]  # [128, 2T]; [p, t] = row t*128+p of this shard
        loss_x[c] = o[:, :_T].T
        loss_y[c] = o[:, _T:].T

    return _host_finish(
        loss_x.reshape(-1), loss_y.reshape(-1), target_weight, use_labels, epoch
    )


# revision 9
# speedup vs baseline: 1.4400x; 1.0008x over previous
"""Trainium2 Bass kernel for nn_Pre_loss_53566832116190 (topk_masking).

Strategy: data-parallel over batch N across 8 NeuronCores. Each core
computes the per-row KL criterion for its shard of rows for both the
(pred_x, gt_x) and (pred_y, gt_y) pairs:

    loss[r] = ( S/s_l - ln(s_l) + ln(s_p) ) / W
      with e_l = exp(l), s_l = sum(e_l), s_p = sum(exp(p)),
           S = sum(e_l * (l - p))

which is algebraically KLDiv(log_softmax(p), softmax(l)).mean(axis=-1).
The small [N*K] merge (global top-k mask, weights, weighted loss sum)
is done on the host after gathering per-core loss rows.
"""

import numpy as np

_N, _K, _W = 1024, 17, 512
_NCORES = 8
_NB = _N // _NCORES      # 128 batch rows per core
_M = _NB * _K            # 2176 loss rows per core
_T = _M // 128           # 17 row-tiles of 128 rows
_COLS = 2 * _T           # x tiles then y tiles

_nc_cache = None
last_results = None  # BassKernelResults of the most recent run (for test harness)


def _build_module():
    import concourse.bacc as bacc
    import concourse.mybir as mybir
    import concourse.tile as tile

    f32 = mybir.dt.float32
    Exp = mybir.ActivationFunctionType.Exp
    Ln = mybir.ActivationFunctionType.Ln
    mult = mybir.AluOpType.mult
    add = mybir.AluOpType.add

    # Bacc (not plain Bass): its finalize() runs generate_event_semaphores,
    # which splits multi-wait sync conditions that TRN2 structs can't hold.
    nc = bacc.Bacc("TRN2", debug=False)

    dram = {}
    for name in ("pred_x", "gt_x", "pred_y", "gt_y"):
        dram[name] = nc.dram_tensor(name, [_M, _W], f32, kind="ExternalInput").ap()
    out_d = nc.dram_tensor("loss_out", [128, _COLS], f32, kind="ExternalOutput").ap()

    # Row-tile groups of up to 4 (1 MiB DMA per tensor per group). Per
    # (group, tensor) the exp row-sums either come from per-row ACT
    # accumulators ("acc") or one grouped DVE reduce ("grp") — the mix
    # balances the ACT and DVE engines.
    groups = []
    t0 = 0
    while t0 < _T:
        gs = min(4, _T - t0)
        groups.append((t0, gs))
        t0 += gs

    def l_mode(gi):
        return "acc" if gi < 3 else "grp"  # 12 of 17 l-tiles per pair: acc

    with tile.TileContext(nc) as tc:
        with (
            tc.tile_pool(name="io", bufs=4) as io_pool,
            tc.tile_pool(name="work", bufs=4) as work_pool,
            tc.tile_pool(name="acc", bufs=1) as acc_pool,
        ):
            # separate accumulator tiles per writer engine so ACT and DVE
            # never write the same tile (avoids cross-engine ordering)
            sl_acc_a = acc_pool.tile([128, _COLS], f32)  # sum exp(l), ACT
            sl_acc_d = acc_pool.tile([128, _COLS], f32)  # sum exp(l), DVE
            sp_acc = acc_pool.tile([128, _COLS], f32)    # sum exp(p), DVE
            ss_acc = acc_pool.tile([128, _COLS], f32)    # sum e_l*(l-p)
            nc.vector.memset(sl_acc_d, 0.0)
            nc.scalar.memzero(sl_acc_a)

            pairs = (("pred_x", "gt_x"), ("pred_y", "gt_y"))
            for pi, (pname, lname) in enumerate(pairs):
                # row r = t*128 + p  ->  [p, t, w]
                p_r = dram[pname].rearrange("(t p) w -> p t w", p=128)
                l_r = dram[lname].rearrange("(t p) w -> p t w", p=128)
                for gi, (t0, gs) in enumerate(groups):
                    lt = io_pool.tile([128, gs, _W], f32, tag="lt")
                    nc.sync.dma_start(out=lt, in_=l_r[:, t0 : t0 + gs, :])
                    pt = io_pool.tile([128, gs, _W], f32, tag="pt")
                    nc.sync.dma_start(out=pt, in_=p_r[:, t0 : t0 + gs, :])

                    col = pi * _T + t0
                    # d = l - p on the (otherwise idle) GpSimd engine, one
                    # row-tile at a time so consumers unblock early
                    dt_ = work_pool.tile([128, gs, _W], f32, tag="dt")
                    for j in range(gs):
                        nc.gpsimd.tensor_sub(
                            dt_[:, j, :], lt[:, j, :], pt[:, j, :]
                        )

                    el = work_pool.tile([128, gs, _W], f32, tag="el")
                    if l_mode(gi) == "acc":
                        for j in range(gs):
                            nc.scalar.activation(
                                el[:, j, :], lt[:, j, :], Exp,
                                accum_out=sl_acc_a[:, col + j : col + j + 1],
                            )
                    else:
                        nc.scalar.activation(el, lt, Exp)
                        nc.vector.tensor_reduce(
                            out=sl_acc_d[:, col : col + gs],
                            in_=el,
                            axis=mybir.AxisListType.X,
                            op=add,
                        )
                    ep = work_pool.tile([128, gs, _W], f32, tag="ep")
                    nc.scalar.activation(ep, pt, Exp)
                    nc.vector.tensor_reduce(
                        out=sp_acc[:, col : col + gs],
                        in_=ep,
                        axis=mybir.AxisListType.X,
                        op=add,
                    )

                    # ss[r] = sum(e_l * d) fused in one DVE pass per row
                    # tile; product scratch overwrites ep (e_p consumed)
                    for j in range(gs):
                        nc.vector.affine_mul_reduce(
                            out=ep[:, j, :],
                            accum_out=ss_acc[:, col + j : col + j + 1],
                            in0=dt_[:, j, :],
                            in1=el[:, j, :],
                            scale=1.0,
                            bias=0.0,
                        )

            sl_acc = acc_pool.tile([128, _COLS], f32)
            nc.vector.tensor_add(sl_acc, sl_acc_a, sl_acc_d)
            res_t = acc_pool.tile([128, _COLS], f32)
            nc.vector.reciprocal(res_t, sl_acc)           # 1/s_l
            nc.vector.tensor_mul(res_t, ss_acc, res_t)    # S/s_l
            ln_sl = acc_pool.tile([128, _COLS], f32)
            nc.scalar.activation(ln_sl, sl_acc, Ln)
            ln_sp = acc_pool.tile([128, _COLS], f32)
            nc.scalar.activation(ln_sp, sp_acc, Ln)
            nc.vector.tensor_sub(res_t, res_t, ln_sl)
            nc.vector.tensor_add(res_t, res_t, ln_sp)
            out_t = acc_pool.tile([128, _COLS], f32)
            nc.scalar.mul(out_t, res_t, 1.0 / _W)
            nc.sync.dma_start(out=out_d, in_=out_t)

    nc.finalize()
    return nc


def get_module():
    global _nc_cache
    if _nc_cache is None:
        _nc_cache = _build_module()
    return _nc_cache


def _host_finish(loss_x, loss_y, target_weight, use_labels, epoch):
    """Replicates reference's cheap [N*K]-sized tail exactly (numpy)."""
    tw = np.asarray(target_weight, dtype=np.float32)
    ul = np.asarray(use_labels)
    weight_real = (tw * (ul == 0).astype(np.float32)[:, None]).reshape(-1)

    cur = float(np.clip(int(epoch) - 210, 0.0, 30.0))
    r = 0.5 * (np.cos(np.pi * cur / 30.0) + 1.0)
    rate = float(np.clip(r, 0.8, 1.0))
    num_visible = int(np.count_nonzero(tw))
    k = int(num_visible * rate)

    tw_flat = tw.reshape(-1)
    loss_all = 0.0
    weights = []
    for loss_small in (loss_x, loss_y):
        loss_new = np.where(tw_flat > 0, loss_small, loss_small.max())
        # k smallest values; ties broken toward lower index (matches
        # jax.lax.top_k on the negated vector)
        idx = np.argsort(loss_new, kind="stable")[:k]
        mask = np.zeros(_N * _K, dtype=np.float32)
        mask[idx] = 1.0
        weight_all = (np.float32(2.0) * weight_real + mask).astype(np.float32)
        weights.append(weight_all)
        loss_all += float(
            np.sum(loss_small.astype(np.float64) * weight_all.astype(np.float64))
        )
    loss = np.float32(loss_all / _K)
    return (np.asarray(loss, dtype=np.float32), weights[0], weights[1])


def _ensure_axon_hooks_importable():
    # concourse.bass_utils imports antenv.axon_hooks when BASS_TRACE is set
    # under axon; some containers ship an antenv stub without it.
    try:
        import antenv.axon_hooks  # noqa: F401
    except Exception:
        import sys
        import types

        m = types.ModuleType("antenv.axon_hooks")
        m._hook = None
        m.set_axon_ntff_profile_hook = lambda hook: setattr(m, "_hook", hook)
        m.get_axon_ntff_profile_hook = lambda: m._hook
        sys.modules["antenv.axon_hooks"] = m


def kernel(pred_x, pred_y, gt_x, gt_y, target_weight, use_labels, epoch):
    global last_results
    _ensure_axon_hooks_importable()
    from concourse import bass_utils

    pred_x = np.ascontiguousarray(np.asarray(pred_x, dtype=np.float32))
    pred_y = np.ascontiguousarray(np.asarray(pred_y, dtype=np.float32))
    gt_x = np.ascontiguousarray(np.asarray(gt_x, dtype=np.float32))
    gt_y = np.ascontiguousarray(np.asarray(gt_y, dtype=np.float32))

    nc = get_module()
    in_maps = []
    for c in range(_NCORES):
        s = slice(c * _NB, (c + 1) * _NB)
        in_maps.append(
            {
                "pred_x": np.ascontiguousarray(pred_x[s].reshape(_M, _W)),
                "gt_x": np.ascontiguousarray(gt_x[s].reshape(_M, _W)),
                "pred_y": np.ascontiguousarray(pred_y[s].reshape(_M, _W)),
                "gt_y": np.ascontiguousarray(gt_y[s].reshape(_M, _W)),
            }
        )

    res = bass_utils.run_bass_kernel_spmd(nc, in_maps, core_ids=list(range(_NCORES)))
    last_results = res

    loss_x = np.empty((_NCORES, _T, 128), dtype=np.float32)
    loss_y = np.empty((_NCORES, _T, 128), dtype=np.float32)
    for c, r in enumerate(res.results):
        o = r["loss_out"]  # [128, 2T]; [p, t] = row t*128+p of this shard
        loss_x[c] = o[:, :_T].T
        loss_y[c] = o[:, _T:].T

    return _host_finish(
        loss_x.reshape(-1), loss_y.reshape(-1), target_weight, use_labels, epoch
    )


# revision 11
# speedup vs baseline: 1.5959x; 1.1083x over previous
"""Trainium2 Bass kernel for nn_Pre_loss_53566832116190 (topk_masking).

Strategy: data-parallel over batch N across 8 NeuronCores. Each core
computes the per-row KL criterion for its shard of rows for both the
(pred_x, gt_x) and (pred_y, gt_y) pairs:

    loss[r] = ( S/s_l - ln(s_l) + ln(s_p) ) / W
      with e_l = exp(l), s_l = sum(e_l), s_p = sum(exp(p)),
           S = sum(e_l * (l - p))

which is algebraically KLDiv(log_softmax(p), softmax(l)).mean(axis=-1).
The small [N*K] merge (global top-k mask, weights, weighted loss sum)
is done on the host after gathering per-core loss rows.
"""

import numpy as np

_N, _K, _W = 1024, 17, 512
_NCORES = 8
_NB = _N // _NCORES      # 128 batch rows per core
_M = _NB * _K            # 2176 loss rows per core
_T = _M // 128           # 17 row-tiles of 128 rows
_COLS = 2 * _T           # x tiles then y tiles

_nc_cache = None
last_results = None  # BassKernelResults of the most recent run (for test harness)


def _build_module():
    import concourse.bacc as bacc
    import concourse.mybir as mybir
    import concourse.tile as tile

    f32 = mybir.dt.float32
    Exp = mybir.ActivationFunctionType.Exp
    Ln = mybir.ActivationFunctionType.Ln
    mult = mybir.AluOpType.mult
    add = mybir.AluOpType.add

    # Bacc (not plain Bass): its finalize() runs generate_event_semaphores,
    # which splits multi-wait sync conditions that TRN2 structs can't hold.
    nc = bacc.Bacc("TRN2", debug=False)

    dram = {}
    for name in ("pred_x", "gt_x", "pred_y", "gt_y"):
        dram[name] = nc.dram_tensor(name, [_M, _W], f32, kind="ExternalInput").ap()
    out_d = nc.dram_tensor("loss_out", [128, _COLS], f32, kind="ExternalOutput").ap()

    # Row-tile groups of up to 4 (1 MiB DMA per tensor per group). Per
    # (group, tensor) the exp row-sums either come from per-row ACT
    # accumulators ("acc") or one grouped DVE reduce ("grp") — the mix
    # balances the ACT and DVE engines.
    groups = []
    t0 = 0
    while t0 < _T:
        gs = min(4, _T - t0)
        groups.append((t0, gs))
        t0 += gs

    # ACT-accum ("acc") vs grouped-exp + DVE reduce ("grp") split per
    # (group, tensor), chosen to balance ACT vs DVE busy time.
    def mode(tens, gi):
        if tens == "l":
            return "acc" if gi < 3 else "grp"
        return "acc" if gi < 2 else "grp"

    with tile.TileContext(nc) as tc:
        with (
            tc.tile_pool(name="io", bufs=4) as io_pool,
            tc.tile_pool(name="work", bufs=4) as work_pool,
            tc.tile_pool(name="acc", bufs=1) as acc_pool,
        ):
            # separate accumulator tiles per writer engine so ACT and DVE
            # never write the same tile (avoids cross-engine ordering)
            sl_acc_a = acc_pool.tile([128, _COLS], f32)  # sum exp(l), ACT
            sl_acc_d = acc_pool.tile([128, _COLS], f32)  # sum exp(l), DVE
            sp_acc_a = acc_pool.tile([128, _COLS], f32)  # sum exp(p), ACT
            sp_acc_d = acc_pool.tile([128, _COLS], f32)  # sum exp(p), DVE
            ss_acc = acc_pool.tile([128, _COLS], f32)    # sum e_l*(l-p)
            nc.vector.memset(sl_acc_d, 0.0)
            nc.vector.memset(sp_acc_d, 0.0)
            nc.scalar.memzero(sl_acc_a)
            nc.scalar.memzero(sp_acc_a)

            pairs = (("pred_x", "gt_x"), ("pred_y", "gt_y"))
            for pi, (pname, lname) in enumerate(pairs):
                # row r = t*128 + p  ->  [p, t, w]
                p_r = dram[pname].rearrange("(t p) w -> p t w", p=128)
                l_r = dram[lname].rearrange("(t p) w -> p t w", p=128)
                for gi, (t0, gs) in enumerate(groups):
                    lt = io_pool.tile([128, gs, _W], f32, tag="lt")
                    nc.sync.dma_start(out=lt, in_=l_r[:, t0 : t0 + gs, :])
                    pt = io_pool.tile([128, gs, _W], f32, tag="pt")
                    nc.sync.dma_start(out=pt, in_=p_r[:, t0 : t0 + gs, :])

                    col = pi * _T + t0
                    dt_ = work_pool.tile([128, gs, _W], f32, tag="dt")
                    nc.vector.tensor_sub(dt_, lt, pt)

                    el = work_pool.tile([128, gs, _W], f32, tag="el")
                    ep = work_pool.tile([128, gs, _W], f32, tag="ep")
                    for tens, src, dst, acc_a, acc_d in (
                        ("l", lt, el, sl_acc_a, sl_acc_d),
                        ("p", pt, ep, sp_acc_a, sp_acc_d),
                    ):
                        if mode(tens, gi) == "acc":
                            for j in range(gs):
                                nc.scalar.activation(
                                    dst[:, j, :], src[:, j, :], Exp,
                                    accum_out=acc_a[:, col + j : col + j + 1],
                                )
                        else:
                            nc.scalar.activation(dst, src, Exp)
                            nc.vector.tensor_reduce(
                                out=acc_d[:, col : col + gs],
                                in_=dst,
                                axis=mybir.AxisListType.X,
                                op=add,
                            )

                    # ss[r] = sum(e_l * d) fused in one DVE pass per row
                    # tile; product scratch overwrites ep (e_p consumed)
                    for j in range(gs):
                        nc.vector.affine_mul_reduce(
                            out=ep[:, j, :],
                            accum_out=ss_acc[:, col + j : col + j + 1],
                            in0=dt_[:, j, :],
                            in1=el[:, j, :],
                            scale=1.0,
                            bias=0.0,
                        )

            sl_acc = acc_pool.tile([128, _COLS], f32)
            nc.vector.tensor_add(sl_acc, sl_acc_a, sl_acc_d)
            sp_acc = acc_pool.tile([128, _COLS], f32)
            nc.vector.tensor_add(sp_acc, sp_acc_a, sp_acc_d)
            res_t = acc_pool.tile([128, _COLS], f32)
            nc.vector.reciprocal(res_t, sl_acc)           # 1/s_l
            nc.vector.tensor_mul(res_t, ss_acc, res_t)    # S/s_l
            ln_sl = acc_pool.tile([128, _COLS], f32)
            nc.scalar.activation(ln_sl, sl_acc, Ln)
            ln_sp = acc_pool.tile([128, _COLS], f32)
            nc.scalar.activation(ln_sp, sp_acc, Ln)
            nc.vector.tensor_sub(res_t, res_t, ln_sl)
            nc.vector.tensor_add(res_t, res_t, ln_sp)
            out_t = acc_pool.tile([128, _COLS], f32)
            nc.scalar.mul(out_t, res_t, 1.0 / _W)
            nc.sync.dma_start(out=out_d, in_=out_t)

    nc.finalize()
    return nc


def get_module():
    global _nc_cache
    if _nc_cache is None:
        _nc_cache = _build_module()
    return _nc_cache


def _host_finish(loss_x, loss_y, target_weight, use_labels, epoch):
    """Replicates reference's cheap [N*K]-sized tail exactly (numpy)."""
    tw = np.asarray(target_weight, dtype=np.float32)
    ul = np.asarray(use_labels)
    weight_real = (tw * (ul == 0).astype(np.float32)[:, None]).reshape(-1)

    cur = float(np.clip(int(epoch) - 210, 0.0, 30.0))
    r = 0.5 * (np.cos(np.pi * cur / 30.0) + 1.0)
    rate = float(np.clip(r, 0.8, 1.0))
    num_visible = int(np.count_nonzero(tw))
    k = int(num_visible * rate)

    tw_flat = tw.reshape(-1)
    loss_all = 0.0
    weights = []
    for loss_small in (loss_x, loss_y):
        loss_new = np.where(tw_flat > 0, loss_small, loss_small.max())
        # k smallest values; ties broken toward lower index (matches
        # jax.lax.top_k on the negated vector)
        idx = np.argsort(loss_new, kind="stable")[:k]
        mask = np.zeros(_N * _K, dtype=np.float32)
        mask[idx] = 1.0
        weight_all = (np.float32(2.0) * weight_real + mask).astype(np.float32)
        weights.append(weight_all)
        loss_all += float(
            np.sum(loss_small.astype(np.float64) * weight_all.astype(np.float64))
        )
    loss = np.float32(loss_all / _K)
    return (np.asarray(loss, dtype=np.float32), weights[0], weights[1])


def _ensure_axon_hooks_importable():
    # concourse.bass_utils imports antenv.axon_hooks when BASS_TRACE is set
    # under axon; some containers ship an antenv stub without it.
    try:
        import antenv.axon_hooks  # noqa: F401
    except Exception:
        import sys
        import types

        m = types.ModuleType("antenv.axon_hooks")
        m._hook = None
        m.set_axon_ntff_profile_hook = lambda hook: setattr(m, "_hook", hook)
        m.get_axon_ntff_profile_hook = lambda: m._hook
        sys.modules["antenv.axon_hooks"] = m


def kernel(pred_x, pred_y, gt_x, gt_y, target_weight, use_labels, epoch):
    global last_results
    _ensure_axon_hooks_importable()
    from concourse import bass_utils

    pred_x = np.ascontiguousarray(np.asarray(pred_x, dtype=np.float32))
    pred_y = np.ascontiguousarray(np.asarray(pred_y, dtype=np.float32))
    gt_x = np.ascontiguousarray(np.asarray(gt_x, dtype=np.float32))
    gt_y = np.ascontiguousarray(np.asarray(gt_y, dtype=np.float32))

    nc = get_module()
    in_maps = []
    for c in range(_NCORES):
        s = slice(c * _NB, (c + 1) * _NB)
        in_maps.append(
            {
                "pred_x": np.ascontiguousarray(pred_x[s].reshape(_M, _W)),
                "gt_x": np.ascontiguousarray(gt_x[s].reshape(_M, _W)),
                "pred_y": np.ascontiguousarray(pred_y[s].reshape(_M, _W)),
                "gt_y": np.ascontiguousarray(gt_y[s].reshape(_M, _W)),
            }
        )

    res = bass_utils.run_bass_kernel_spmd(nc, in_maps, core_ids=list(range(_NCORES)))
    last_results = res

    loss_x = np.empty((_NCORES, _T, 128), dtype=np.float32)
    loss_y = np.empty((_NCORES, _T, 128), dtype=np.float32)
    for c, r in enumerate(res.results):
        o = r["loss_out"]  # [128, 2T]; [p, t] = row t*128+p of this shard
        loss_x[c] = o[:, :_T].T
        loss_y[c] = o[:, _T:].T

    return _host_finish(
        loss_x.reshape(-1), loss_y.reshape(-1), target_weight, use_labels, epoch
    )


# revision 15
# speedup vs baseline: 1.7039x; 1.0676x over previous
"""Trainium2 Bass kernel for nn_Pre_loss_53566832116190 (topk_masking).

Strategy: data-parallel over batch N across 8 NeuronCores. Each core
computes the per-row KL criterion for its shard of rows for both the
(pred_x, gt_x) and (pred_y, gt_y) pairs:

    loss[r] = ( S/s_l - ln(s_l) + ln(s_p) ) / W
      with e_l = exp(l), s_l = sum(e_l), s_p = sum(exp(p)),
           S = sum(e_l * (l - p))

which is algebraically KLDiv(log_softmax(p), softmax(l)).mean(axis=-1).
The small [N*K] merge (global top-k mask, weights, weighted loss sum)
is done on the host after gathering per-core loss rows.
"""

import numpy as np

_N, _K, _W = 1024, 17, 512
_NCORES = 8
_NB = _N // _NCORES      # 128 batch rows per core
_M = _NB * _K            # 2176 loss rows per core
_T = _M // 128           # 17 row-tiles of 128 rows
_COLS = 2 * _T           # x tiles then y tiles

_nc_cache = None
last_results = None  # BassKernelResults of the most recent run (for test harness)


def _build_module():
    import concourse.bacc as bacc
    import concourse.mybir as mybir
    import concourse.tile as tile

    f32 = mybir.dt.float32
    Exp = mybir.ActivationFunctionType.Exp
    Ln = mybir.ActivationFunctionType.Ln
    mult = mybir.AluOpType.mult
    add = mybir.AluOpType.add

    # Bacc (not plain Bass): its finalize() runs generate_event_semaphores,
    # which splits multi-wait sync conditions that TRN2 structs can't hold.
    nc = bacc.Bacc("TRN2", debug=False)

    dram = {}
    for name in ("pred_x", "gt_x", "pred_y", "gt_y"):
        dram[name] = nc.dram_tensor(name, [_M, _W], f32, kind="ExternalInput").ap()
    out_d = nc.dram_tensor("loss_out", [128, _COLS], f32, kind="ExternalOutput").ap()

    # Row-tile groups of up to 4 (1 MiB DMA per tensor per group). Per
    # (group, tensor) the exp row-sums either come from per-row ACT
    # accumulators ("acc") or one grouped DVE reduce ("grp") — the mix
    # balances the ACT and DVE engines.
    groups = []
    t0 = 0
    while t0 < _T:
        gs = min(4, _T - t0)
        groups.append((t0, gs))
        t0 += gs

    # ACT-accum ("acc") vs grouped-exp + DVE reduce ("grp") split per
    # (group, tensor), chosen to balance ACT vs DVE busy time.
    def mode(tens, gi):
        if tens == "l":
            return "acc"
        return "acc" if gi < 2 else "grp"

    with tile.TileContext(nc) as tc:
        with (
            tc.tile_pool(name="io", bufs=5) as io_pool,
            tc.tile_pool(name="work", bufs=4) as work_pool,
            tc.tile_pool(name="acc", bufs=1) as acc_pool,
        ):
            # separate accumulator tiles per writer engine so ACT and DVE
            # never write the same tile (avoids cross-engine ordering)
            sl_acc_a = acc_pool.tile([128, _COLS], f32)  # sum exp(l), ACT
            sl_acc_d = acc_pool.tile([128, _COLS], f32)  # sum exp(l), DVE
            sp_acc_a = acc_pool.tile([128, _COLS], f32)  # sum exp(p), ACT
            sp_acc_d = acc_pool.tile([128, _COLS], f32)  # sum exp(p), DVE
            ss_acc = acc_pool.tile([128, _COLS], f32)    # sum e_l*(l-p)
            nc.vector.memset(sl_acc_d, 0.0)
            nc.vector.memset(sp_acc_d, 0.0)
            nc.scalar.memzero(sl_acc_a)
            nc.scalar.memzero(sp_acc_a)

            pairs = (("pred_x", "gt_x"), ("pred_y", "gt_y"))
            for pi, (pname, lname) in enumerate(pairs):
                # partition-major rows: r = p*T + t -> [p, t, w]; each
                # partition reads contiguous DRAM (T rows back-to-back)
                p_r = dram[pname].rearrange("(p t) w -> p t w", t=_T)
                l_r = dram[lname].rearrange("(p t) w -> p t w", t=_T)
                for gi, (t0, gs) in enumerate(groups):
                    lt = io_pool.tile([128, gs, _W], f32, tag="lt")
                    nc.sync.dma_start(out=lt, in_=l_r[:, t0 : t0 + gs, :])
                    pt = io_pool.tile([128, gs, _W], f32, tag="pt")
                    nc.sync.dma_start(out=pt, in_=p_r[:, t0 : t0 + gs, :])

                    col = pi * _T + t0
                    dt_ = work_pool.tile([128, gs, _W], f32, tag="dt")
                    nc.vector.tensor_sub(dt_, lt, pt)

                    el = work_pool.tile([128, gs, _W], f32, tag="el")
                    ep = work_pool.tile([128, gs, _W], f32, tag="ep")
                    for tens, src, dst, acc_a, acc_d in (
                        ("l", lt, el, sl_acc_a, sl_acc_d),
                        ("p", pt, ep, sp_acc_a, sp_acc_d),
                    ):
                        if mode(tens, gi) == "acc":
                            for j in range(gs):
                                nc.scalar.activation(
                                    dst[:, j, :], src[:, j, :], Exp,
                                    accum_out=acc_a[:, col + j : col + j + 1],
                                )
                        else:
                            nc.scalar.activation(dst, src, Exp)
                            nc.vector.tensor_reduce(
                                out=acc_d[:, col : col + gs],
                                in_=dst,
                                axis=mybir.AxisListType.X,
                                op=add,
                            )

                    # ss[r] = sum(e_l * d) fused in one DVE pass per row
                    # tile; product scratch overwrites ep (e_p consumed)
                    for j in range(gs):
                        nc.vector.affine_mul_reduce(
                            out=ep[:, j, :],
                            accum_out=ss_acc[:, col + j : col + j + 1],
                            in0=dt_[:, j, :],
                            in1=el[:, j, :],
                            scale=1.0,
                            bias=0.0,
                        )

            sl_acc = acc_pool.tile([128, _COLS], f32)
            nc.vector.tensor_add(sl_acc, sl_acc_a, sl_acc_d)
            sp_acc = acc_pool.tile([128, _COLS], f32)
            nc.vector.tensor_add(sp_acc, sp_acc_a, sp_acc_d)
            res_t = acc_pool.tile([128, _COLS], f32)
            nc.vector.reciprocal(res_t, sl_acc)           # 1/s_l
            nc.vector.tensor_mul(res_t, ss_acc, res_t)    # S/s_l
            ln_sl = acc_pool.tile([128, _COLS], f32)
            nc.scalar.activation(ln_sl, sl_acc, Ln)
            ln_sp = acc_pool.tile([128, _COLS], f32)
            nc.scalar.activation(ln_sp, sp_acc, Ln)
            nc.vector.tensor_sub(res_t, res_t, ln_sl)
            nc.vector.tensor_add(res_t, res_t, ln_sp)
            out_t = acc_pool.tile([128, _COLS], f32)
            nc.scalar.mul(out_t, res_t, 1.0 / _W)
            nc.sync.dma_start(out=out_d, in_=out_t)

    nc.finalize()
    return nc


def get_module():
    global _nc_cache
    if _nc_cache is None:
        _nc_cache = _build_module()
    return _nc_cache


def _host_finish(loss_x, loss_y, target_weight, use_labels, epoch):
    """Replicates reference's cheap [N*K]-sized tail exactly (numpy)."""
    tw = np.asarray(target_weight, dtype=np.float32)
    ul = np.asarray(use_labels)
    weight_real = (tw * (ul == 0).astype(np.float32)[:, None]).reshape(-1)

    cur = float(np.clip(int(epoch) - 210, 0.0, 30.0))
    r = 0.5 * (np.cos(np.pi * cur / 30.0) + 1.0)
    rate = float(np.clip(r, 0.8, 1.0))
    num_visible = int(np.count_nonzero(tw))
    k = int(num_visible * rate)

    tw_flat = tw.reshape(-1)
    loss_all = 0.0
    weights = []
    for loss_small in (loss_x, loss_y):
        loss_new = np.where(tw_flat > 0, loss_small, loss_small.max())
        # k smallest values; ties broken toward lower index (matches
        # jax.lax.top_k on the negated vector)
        idx = np.argsort(loss_new, kind="stable")[:k]
        mask = np.zeros(_N * _K, dtype=np.float32)
        mask[idx] = 1.0
        weight_all = (np.float32(2.0) * weight_real + mask).astype(np.float32)
        weights.append(weight_all)
        loss_all += float(
            np.sum(loss_small.astype(np.float64) * weight_all.astype(np.float64))
        )
    loss = np.float32(loss_all / _K)
    return (np.asarray(loss, dtype=np.float32), weights[0], weights[1])


def _ensure_axon_hooks_importable():
    # concourse.bass_utils imports antenv.axon_hooks when BASS_TRACE is set
    # under axon; some containers ship an antenv stub without it.
    try:
        import antenv.axon_hooks  # noqa: F401
    except Exception:
        import sys
        import types

        m = types.ModuleType("antenv.axon_hooks")
        m._hook = None
        m.set_axon_ntff_profile_hook = lambda hook: setattr(m, "_hook", hook)
        m.get_axon_ntff_profile_hook = lambda: m._hook
        sys.modules["antenv.axon_hooks"] = m


def kernel(pred_x, pred_y, gt_x, gt_y, target_weight, use_labels, epoch):
    global last_results
    _ensure_axon_hooks_importable()
    from concourse import bass_utils

    pred_x = np.ascontiguousarray(np.asarray(pred_x, dtype=np.float32))
    pred_y = np.ascontiguousarray(np.asarray(pred_y, dtype=np.float32))
    gt_x = np.ascontiguousarray(np.asarray(gt_x, dtype=np.float32))
    gt_y = np.ascontiguousarray(np.asarray(gt_y, dtype=np.float32))

    nc = get_module()
    in_maps = []
    for c in range(_NCORES):
        s = slice(c * _NB, (c + 1) * _NB)
        in_maps.append(
            {
                "pred_x": np.ascontiguousarray(pred_x[s].reshape(_M, _W)),
                "gt_x": np.ascontiguousarray(gt_x[s].reshape(_M, _W)),
                "pred_y": np.ascontiguousarray(pred_y[s].reshape(_M, _W)),
                "gt_y": np.ascontiguousarray(gt_y[s].reshape(_M, _W)),
            }
        )

    res = bass_utils.run_bass_kernel_spmd(nc, in_maps, core_ids=list(range(_NCORES)))
    last_results = res

    loss_x = np.empty((_NCORES, 128, _T), dtype=np.float32)
    loss_y = np.empty((_NCORES, 128, _T), dtype=np.float32)
    for c, r in enumerate(res.results):
        o = r["loss_out"]  # [128, 2T]; [p, t] = row p*T+t of this shard
        loss_x[c] = o[:, :_T]
        loss_y[c] = o[:, _T:]

    return _host_finish(
        loss_x.reshape(-1), loss_y.reshape(-1), target_weight, use_labels, epoch
    )
